# revision 20
# baseline (speedup 1.0000x reference)
"""Trainium2 Bass kernel for nn_Biaffine (B=4, S=512, D=512, R=64).

Math: the reference computes
    left = einsum('bxi,irj,byj->bxyr', hf, U1, hb)
    out  = mean_y(left + rf[:, :, None] + rb[:, None] + bias)
The mean over y commutes with everything:
    mean_y(left)[b,x,r] = sum_ij hf[b,x,i] U1[i,r,j] hbbar[b,j],
    hbbar = mean_y(hb).
So out[b,x,r] = sum_i hf[b,x,i] * (V[b,i,r] + U2a[i,r]) + rbbar[b,r] + bias[r]
with V[b,i,r] = sum_j U1[i,r,j] hbbar[b,j], rbbar = hbbar @ U2b.

Sharding: tensor-parallel over r (dep_vec_dim): core c owns r in [8c, 8c+8).
Each core reads its U1 shard (fp8e4m3, x1024-scaled host-side; measured rel
err 6e-3 vs the 2e-2 gate since the fp32-path rf term dominates the output),
plus the full hf and hb in fp16. hb's mean is computed locally per core —
no collectives (the previous ncfw AllReduce had a ~20us latency floor), so
the 8 cores run fully decoupled. Per-core HBM traffic ~6.2MB/execution.

Compute layout (the big win vs the r-moving formulation, 46us -> ~29us):
U1 is the PE-stationary operand as [j, i] 128x128 tiles with hbbar moving,
so V comes out of the PE already on i-partitions ([i, (ic,r,b)] in one PSUM
bank) — no PE transposes, no PSUM->SBUF->PE round-trips, and fp8 weights
load at FWL rate. The final matmul makes hf stationary ([i, x] tiles) with
V+U2a moving, so out lands x-major ([p, b, xc, r], one contiguous 512B run
per partition, reordered on host) and rbbar+bias ride in via a ones-column
broadcast matmul. Loads are split across both HWDGE rings (SP: hb/2 + u1,
ACT: hb/2 + hft); the single merged out store rides SWDGE so it never
head-of-line-blocks the next execution's loads in the pipelined repeat
measurement.
"""

import os
import sys

import numpy as np

try:
    import concourse.bass as bass  # noqa: F401
except ImportError:  # pragma: no cover
    sys.path.insert(0, "/opt/trn_rl_repo")

B, S, D, R = 4, 512, 512, 64
NCORES = 8
RB = R // NCORES  # 8 r's per core
P = 128
JC = D // P  # 4 j-chunks
IC = D // P  # 4 i-chunks
XC = S // P  # 4 x-chunks

# U1_MODE: "fp8" = u1 fp8 x1024, hbb fp16 /32;  "fp16" = u1 fp16 x1, hbb fp16;
# "fp8both" = u1 fp8 x64, hbb fp8 x16
U1_MODE = os.environ.get("U1_MODE", "fp8")
HB_MODE = os.environ.get("HB_MODE", "fp16")
if U1_MODE == "fp8":
    U1_SCALE, HBB_SCALE = 1024.0, 1.0 / 32.0
elif U1_MODE == "fp8both":
    U1_SCALE, HBB_SCALE = 64.0, 16.0
else:
    U1_SCALE, HBB_SCALE = 1.0, 1.0
V_DESCALE = U1_SCALE * HBB_SCALE  # ps_v = V * this

TRACE = os.environ.get("BASS_KERNEL_TRACE", "0") == "1"
LAST_RESULTS = None

_NC_CACHE = {}


def _build_nc(n_repeat=1, variant="full"):
    import concourse.bacc as bacc
    import concourse.mybir as mybir
    import concourse.tile as tile
    fp32 = mybir.dt.float32
    fp16 = mybir.dt.float16
    fp8 = mybir.dt.float8e4
    u1dt = fp16 if U1_MODE == "fp16" else fp8
    hbbdt = fp8 if U1_MODE == "fp8both" else fp16
    hbdt = fp8 if HB_MODE == "fp8" else fp16

    nc = bacc.Bacc("TRN2", target_bir_lowering=False, debug=False, num_devices=NCORES)

    hft_d = nc.dram_tensor("hft", [B, D, S], fp16, kind="ExternalInput")
    hb_d = nc.dram_tensor("hb", [D, B, S], hbdt, kind="ExternalInput")
    u1t_d = nc.dram_tensor("u1t", [D, RB, D], u1dt, kind="ExternalInput")
    u2t_d = nc.dram_tensor("u2t", [P, IC, 2 * RB], fp16, kind="ExternalInput")
    bias_d = nc.dram_tensor("biasbt", [1, B * RB], fp16, kind="ExternalInput")
    # SBUF-native layout: one 512B-contiguous run per partition; host reorders
    out_d = nc.dram_tensor("out", [P, B, XC, RB], fp32, kind="ExternalOutput")

    with tile.TileContext(nc) as tc:
        with (
            tc.tile_pool(name="const", bufs=1) as cpool,
            tc.tile_pool(name="data", bufs=1) as dpool,
            tc.tile_pool(name="psum", bufs=8, space="PSUM") as ppool,
        ):
            ones16 = cpool.tile([1, P], fp16, tag="ones16")
            nc.vector.memset(ones16, 1.0)

            cst = None
            if variant == "compute":
                c_hbt = cpool.tile([P, JC, B, S], hbdt, tag="c_hb")
                nc.vector.memset(c_hbt, 0.01)
                c_u1 = []
                for jc in range(JC):
                    t = cpool.tile([P, RB, D], u1dt, tag=f"c_u1_{jc}")
                    nc.vector.memset(t, 0.01)
                    c_u1.append(t)
                c_hft = []
                for b in range(B):
                    t = cpool.tile([P, IC, S], fp16, tag=f"c_hft{b}")
                    nc.vector.memset(t, 0.01)
                    c_hft.append(t)
                c_u2 = cpool.tile([P, IC, 2 * RB], fp16, tag="c_u2")
                nc.vector.memset(c_u2, 0.01)
                c_bias = cpool.tile([1, B * RB], fp16, tag="c_bias")
                nc.vector.memset(c_bias, 0.0)
                cst = (c_hbt, c_u1, c_hft, c_u2, c_bias)

            for _rep in range(n_repeat):
                _emit_body(
                    nc, dpool, ppool, fp32, fp16, u1dt, hbbdt, hbdt, ones16,
                    hft_d, hb_d, u1t_d, u2t_d, bias_d, out_d,
                    variant=variant, cst=cst,
                )

    nc.compile()
    return nc


def _emit_body(
    nc, dpool, ppool, fp32, fp16, u1dt, hbbdt, hbdt, ones16,
    hft_d, hb_d, u1t_d, u2t_d, bias_d, out_d,
    variant="full", cst=None,
):
    import concourse.mybir as mybir

    if variant == "compute":
        hbt, u1_tiles, hft_tiles, u2sb, bias_sb = cst
    else:
        u2sb = dpool.tile([P, IC, 2 * RB], fp16, tag="u2sb", bufs=2)
        bias_sb = dpool.tile([1, B * RB], fp16, tag="bias_sb", bufs=2)

        # --- small inputs ---
        nc.sync.dma_start(out=u2sb, in_=u2t_d.ap())
        nc.sync.dma_start(out=bias_sb, in_=bias_d.ap())

        # --- full hb (fp16, host-transposed to [j, b, y]); the mean is a
        # DVE free-axis reduce per (jc, b)
        hbt = dpool.tile([P, JC, B, S], hbdt, tag="hb", bufs=3)
        hb_ap = hb_d.ap().rearrange("(jc p) b y -> p jc b y", p=P)
        nc.sync.dma_start(out=hbt[:, : JC // 2], in_=hb_ap[:, : JC // 2])
        nc.scalar.dma_start(out=hbt[:, JC // 2 :], in_=hb_ap[:, JC // 2 :])

        # --- big loads issued up-front, split across the two HWDGE rings
        # (SP carries hb+u1, ACT carries hft); out stores ride SWDGE so
        # they never head-of-line-block the next rep's loads.
        u1_tiles = []
        for jc in range(JC):
            u1t_t = dpool.tile([P, RB, D], u1dt, tag=f"u1_{jc}", bufs=3)
            nc.sync.dma_start(out=u1t_t, in_=u1t_d.ap()[jc * P : (jc + 1) * P])
            u1_tiles.append(u1t_t)
        hft_tiles = []
        for b in range(B):
            hft_t = dpool.tile([P, IC, S], fp16, tag=f"hft{b}", bufs=3)
            nc.scalar.dma_start(
                out=hft_t, in_=hft_d.ap()[b].rearrange("(ic p) x -> p ic x", p=P)
            )
            hft_tiles.append(hft_t)

    if variant == "dma":
        out_sb = dpool.tile([P, B, XC, RB], fp32, tag="out", bufs=2)
        nc.vector.memset(out_sb, 0.5)
        nc.gpsimd.dma_start(out=out_d.ap(), in_=out_sb)
        return

    hbbsum = dpool.tile([P, JC * B], fp16, tag="hbbsum", bufs=2)
    hbb16 = dpool.tile([P, JC * B], hbbdt, tag="hbb16", bufs=2)
    vass = dpool.tile([P, IC, B, RB], fp16, tag="vass", bufs=2)

    # hbbsum[j, (jc,b)] = sum_y hb[j, b, y] (unscaled; u2b carries 1/S),
    # via DVE free-axis reduces; hbb16 = hbbsum/(S*32) feeds the V matmuls
    with nc.allow_low_precision(reason="fp16 y-sum; |sum|<~60, fp32-internal"):
        for b in range(B):
            for jc in range(JC):
                nc.vector.reduce_sum(
                    hbbsum[:, jc * B + b : jc * B + b + 1],
                    hbt[:, jc, b, :],
                    axis=mybir.AxisListType.X,
                )
    nc.vector.tensor_scalar(
        out=hbb16, in0=hbbsum, scalar1=float(HBB_SCALE / S),
        scalar2=None, op0=mybir.AluOpType.mult,
    )

    # --- rbbar flat on one PSUM row: rbbar[b,r] = sum_j hbbsum[j,b]*u2b[j,r]
    # (M=1 matmuls, K=j), then broadcast to all 128 partitions with a
    # ones-column matmul, adding the host-tiled bias row in the same group.
    ps_rb = ppool.tile([P, 512], fp32, tag="ps")
    for b in range(B):
        for jc in range(JC):
            nc.tensor.matmul(
                ps_rb[:1, b * RB : (b + 1) * RB],
                hbbsum[:, jc * B + b : jc * B + b + 1],
                u2sb[:, jc, RB : 2 * RB],
                start=(jc == 0),
                stop=(jc == JC - 1),
            )
    rbbflat = dpool.tile([1, B * RB], fp16, tag="rbbflat", bufs=2)
    nc.vector.tensor_copy(out=rbbflat, in_=ps_rb[:1, : B * RB])
    ps_bx = ppool.tile([P, 512], fp32, tag="ps")
    nc.tensor.matmul(
        ps_bx[:, : B * RB], ones16, rbbflat, start=True, stop=False
    )
    nc.tensor.matmul(
        ps_bx[:, : B * RB], ones16, bias_sb, start=False, stop=True
    )
    rbbx = dpool.tile([P, B, RB], fp16, tag="rbbx", bufs=2)
    nc.vector.tensor_copy(
        out=rbbx, in_=ps_bx[:, : B * RB].rearrange("p (b r) -> p b r", r=RB)
    )

    # --- V on i-partitions: lhsT = U1 [j, i] 128x128 fp8 stationary tiles,
    # rhs = hbb16 [j, b] moving; out ps_v[i, (ic, r, b)] one PSUM bank.
    ps_v = ppool.tile([P, 512], fp32, tag="ps")
    for ic in range(IC):
        for r in range(RB):
            for jc in range(JC):
                nc.tensor.matmul(
                    ps_v[:, (ic * RB + r) * B : (ic * RB + r) * B + B],
                    u1_tiles[jc][:, r, ic * P : (ic + 1) * P],
                    hbb16[:, jc * B : (jc + 1) * B],
                    start=(jc == 0),
                    stop=(jc == JC - 1),
                )
    # vass[i, ic, b, r] = ps_v/32 + U2a[i, r]  (two DVE sweeps, 128 el/lane)
    vtmp = dpool.tile([P, IC, RB, B], fp16, tag="vtmp", bufs=2)
    nc.vector.tensor_scalar(
        out=vtmp,
        in0=ps_v[:, : IC * RB * B].rearrange(
            "p (ic r b) -> p ic r b", r=RB, b=B
        ),
        scalar1=float(1.0 / V_DESCALE), scalar2=None,
        op0=mybir.AluOpType.mult,
    )
    nc.vector.tensor_tensor(
        out=vass,
        in0=vtmp.rearrange("p ic r b -> p ic b r"),
        in1=u2sb[:, :, None, :RB].to_broadcast((P, IC, B, RB)),
        op=mybir.AluOpType.add,
    )

    # --- out[x, r] per (b, xc): lhsT = hft [i, x] 128x128 fp16 stationary,
    # rhs = vass [i, r] moving; accumulate over ic ---
    ps_o = ppool.tile([P, 512], fp32, tag="ps")
    for b in range(B):
        for xc in range(XC):
            for ic in range(IC):
                nc.tensor.matmul(
                    ps_o[:, (b * XC + xc) * RB : (b * XC + xc + 1) * RB],
                    hft_tiles[b][:, ic, xc * P : (xc + 1) * P],
                    vass[:, ic, b, :],
                    start=(ic == 0),
                    stop=(ic == IC - 1),
                )
    out_sb = dpool.tile([P, B, XC, RB], fp32, tag="out", bufs=2)
    nc.vector.tensor_tensor(
        out=out_sb,
        in0=ps_o[:, : B * XC * RB].rearrange(
            "p (b xc r) -> p b xc r", xc=XC, r=RB
        ),
        in1=rbbx[:, :, None, :].to_broadcast((P, B, XC, RB)),
        op=mybir.AluOpType.add,
    )
    nc.gpsimd.dma_start(out=out_d.ap(), in_=out_sb)


def _get_nc(n_repeat=1, variant="full"):
    key = (n_repeat, variant)
    if key not in _NC_CACHE:
        _NC_CACHE[key] = _build_nc(n_repeat, variant)
    return _NC_CACHE[key]


def _prep_inputs(h_forward, h_backward, U_1, U_2, bias):
    import ml_dtypes

    hf = np.asarray(h_forward, dtype=np.float32)
    hb = np.asarray(h_backward, dtype=np.float32)
    u1 = np.asarray(U_1, dtype=np.float32)
    u2 = np.asarray(U_2, dtype=np.float32)
    bz = np.asarray(bias, dtype=np.float32)

    hft = np.ascontiguousarray(hf.transpose(0, 2, 1).astype(np.float16))  # [B, i, x]
    hbdt_np = ml_dtypes.float8_e4m3 if HB_MODE == "fp8" else np.float16
    hbt = np.ascontiguousarray(hb.transpose(2, 0, 1).astype(hbdt_np))  # [j, b, y]

    in_maps = []
    for c in range(NCORES):
        rs = slice(c * RB, (c + 1) * RB)
        u1dt_np = np.float16 if U1_MODE == "fp16" else ml_dtypes.float8_e4m3
        u1t_c = np.ascontiguousarray(
            (u1[:, rs, :].transpose(2, 1, 0) * np.float32(U1_SCALE)).astype(
                u1dt_np
            )
        )  # [j, r, i]
        # pre-packed u2sb layout [d%P, dchunk, 2*RB]: cols 0:RB = U2a[d, rs],
        # RB:2RB = U2b[d, rs] * 1/S (hbbsum is an unscaled sum over y)
        u2t_c = np.ascontiguousarray(
            np.concatenate(
                [
                    u2[:D, rs].reshape(IC, P, RB).transpose(1, 0, 2),
                    u2[D:, rs].reshape(IC, P, RB).transpose(1, 0, 2)
                    * np.float32(1.0 / S),
                ],
                axis=2,
            ).astype(np.float16)
        )
        bias_c = np.ascontiguousarray(
            np.tile(bz[rs], B).reshape(1, B * RB).astype(np.float16)
        )
        in_maps.append(
            {
                "hft": hft,
                "hb": hbt,
                "u1t": u1t_c,
                "u2t": u2t_c,
                "biasbt": bias_c,
            }
        )
    return in_maps


def _get_exec():
    """One jitted sharded executable, cached for the process lifetime."""
    if "exec" in _EXEC_CACHE:
        return _EXEC_CACHE["exec"]

    import jax
    from jax.sharding import Mesh, PartitionSpec

    import warnings

    with warnings.catch_warnings():
        warnings.simplefilter("ignore")
        from jax.experimental.shard_map import shard_map

    from concourse import mybir
    from concourse.bass2jax import (
        _bass_exec_p,
        install_neuronx_cc_hook,
        partition_id_tensor,
    )

    install_neuronx_cc_hook()
    nc = _get_nc()
    partition_name = nc.partition_id_tensor.name if nc.partition_id_tensor else None
    in_names, out_names, out_avals = [], [], []
    for alloc in nc.m.functions[0].allocations:
        if not isinstance(alloc, mybir.MemoryLocationSet):
            continue
        name = alloc.memorylocations[0].name
        if alloc.kind == "ExternalInput":
            if name != partition_name:
                in_names.append(name)
        elif alloc.kind == "ExternalOutput":
            out_names.append(name)
            out_avals.append(
                jax.core.ShapedArray(tuple(alloc.tensor_shape), mybir.dt.np(alloc.dtype))
            )
    all_names = in_names + out_names
    if partition_name is not None:
        all_names = all_names + [partition_name]

    def _body(*args):
        operands = list(args)
        if partition_name is not None:
            operands.append(partition_id_tensor())
        return tuple(
            _bass_exec_p.bind(
                *operands,
                out_avals=tuple(out_avals),
                in_names=tuple(all_names),
                out_names=tuple(out_names),
                lowering_input_output_aliases=(),
                sim_require_finite=True,
                sim_require_nnan=True,
                nc=nc,
            )
        )

    devices = jax.devices()[:NCORES]
    mesh = Mesh(np.asarray(devices), ("core",))
    n_args = len(in_names) + len(out_avals)
    fn = jax.jit(
        shard_map(
            _body,
            mesh=mesh,
            in_specs=(PartitionSpec("core"),) * n_args,
            out_specs=(PartitionSpec("core"),) * len(out_names),
            check_rep=False,
        ),
        keep_unused=True,
    )
    sh = jax.sharding.NamedSharding(mesh, PartitionSpec("core"))
    _EXEC_CACHE["exec"] = (fn, sh, in_names, out_names, out_avals)
    return _EXEC_CACHE["exec"]


_EXEC_CACHE = {}


def kernel(h_forward, h_backward, U_1, U_2, bias):
    import jax

    fn, sh, in_names, out_names, out_avals = _get_exec()
    in_maps = _prep_inputs(h_forward, h_backward, U_1, U_2, bias)
    args = [
        jax.device_put(
            np.concatenate([in_maps[c][name] for c in range(NCORES)], axis=0), sh
        )
        for name in in_names
    ]
    for av in out_avals:
        args.append(
            jax.device_put(
                np.zeros((NCORES * av.shape[0], *av.shape[1:]), av.dtype), sh
            )
        )
    out_arrs = fn(*args)
    oi = out_names.index("out")
    full = np.asarray(out_arrs[oi]).reshape(NCORES, P, B, XC, RB)
    # [core, p, b, xc, r] -> [b, xc, p, core, r] -> [B, S, R]
    return np.ascontiguousarray(
        full.transpose(2, 3, 1, 0, 4).reshape(B, S, R)
    )
